# revision 20
# baseline (speedup 1.0000x reference)
"""Trainium2 Bass kernel for nn_EncoderLayer (dense transformer encoder layer
with static-expansion attention-like block + FF), data-parallel over 8 cores.

Contract: kernel(**inputs) takes FULL unsharded inputs (as in setup_inputs()),
returns the FULL (64, 256, 512) float32 output.
"""

import sys

for _p in ("/opt/trn_rl_repo",):
    if _p not in sys.path:
        sys.path.insert(0, _p)

import numpy as np

import concourse.bass as bass
import concourse.mybir as mybir
import concourse.tile as tile
from concourse.vector_clock import ScopedClock

F32 = mybir.dt.float32
F32R = mybir.dt.float32r
I32 = mybir.dt.int32
AX = mybir.AxisListType
OP = mybir.AluOpType
AF = mybir.ActivationFunctionType

D = 512          # d_model
DFF = 2048       # d_ff
N = 992          # n experts
L = 256          # enc len
BS = 64
NCORES = 8
BPC = BS // NCORES  # batch elements per core
EPS = 1e-9
LN_EPS = 1e-5

KD = D // 128     # 4 k-chunks over d_model
LT = L // 128     # 2 l-chunks
NMO = 8           # n-chunks over N (7x128 + 96)
NSZ = [128] * 7 + [96]
NOFF = [128 * i for i in range(8)]
KF = DFF // 128   # 16 chunks over d_ff


class SplitDrainTC(tile.TileContext):
    """TileContext whose exit drain splits semaphore waits across nop
    instructions (this walrus build rejects >2 sync waits on one Drain)."""

    def _drain_and_barrier(self, tick_clock, wait_clock):
        nc = self.nc
        probe = nc.sync.nop(nofuse=True)
        wait_clock.add_sem_waits(probe.ins, ScopedClock({None: tick_clock.global_clock}))
        si = probe.ins.sync_info
        waits = list(si.on_wait) if si and si.on_wait else []
        if len(waits) > 1:
            si.on_wait = waits[:1]
            sems_by_name = {h.name: h for h in self.sems.allocated().values()}
            for w in waits[1:]:
                n2 = nc.sync.nop(nofuse=True)
                n2.wait_op(sems_by_name[w.ant_name], w.wait_value, "sem-ge")
        nc.sync.drain()
        nc.all_engine_barrier()
        popped = nc._tile_sem_poison_stack.pop()
        assert popped is self._sem_poison
        nc.clear_and_free_semaphores(list(self.sems.allocated().values()))
        nc.all_engine_barrier()


def _r(ap):
    return ap.bitcast(F32R)


def _split_excess_waits(nc, cap=2):
    """This walrus build rejects instructions carrying more than ~2 sync
    waits. Hoist excess waits onto same-engine nop instructions inserted
    immediately before the offending instruction (engine program order is
    bb order, so the nop's waits complete first)."""
    import bass_rust
    for f in nc.m.functions:
        for bb in f.blocks:
            over = [inst for inst in bb.instructions
                    if inst.sync_info and inst.sync_info.on_wait
                    and len(inst.sync_info.on_wait) > cap]
            if not over:
                continue
            carriers = {}
            for inst in over:
                waits = list(inst.sync_info.on_wait)
                inst.sync_info.on_wait = waits[:cap]
                rest = waits[cap:]
                lst = []
                for i in range(0, len(rest), cap):
                    nop = nc.engines[inst.engine].nop(nofuse=True)
                    cur = nc.cur_bb.bb
                    assert cur.instructions[-1] is nop.ins
                    cur.instructions.pop()
                    nop.ins.sync_info = bass_rust.SyncInfo(
                        on_wait=rest[i:i + cap], on_update=[])
                    lst.append(nop.ins)
                carriers[inst.name] = lst
            out = []
            for inst in bb.instructions:
                out.extend(carriers.get(inst.name, ()))
                out.append(inst)
            bb.instructions[:] = out


def build_program(n_elems=BPC):
    """Single-core SPMD program; see kernel() for the per-core input map."""
    nc = bass.Bass("TRN2", target_bir_lowering=False, debug=False)

    x_d = nc.dram_tensor("x", [n_elems, L, D], F32, kind="ExternalInput").ap()
    nidx_d = nc.dram_tensor("nidx", [n_elems, N], I32, kind="ExternalInput").ap()
    mask_d = nc.dram_tensor("mask", [n_elems, N, L], I32, kind="ExternalInput").ap()
    btab_d = nc.dram_tensor("b_tab", [N, D], F32, kind="ExternalInput").ap()
    qT_d = nc.dram_tensor("qT", [D, N], F32R, kind="ExternalInput").ap()
    w6_d = nc.dram_tensor("w6", [6, D, D], F32R, kind="ExternalInput").ap()
    brows_d = nc.dram_tensor("brows", [6, D], F32R, kind="ExternalInput").ap()
    bkrow_d = nc.dram_tensor("bkrow", [D], F32, kind="ExternalInput").ap()
    wf1_d = nc.dram_tensor("wf1", [D, DFF], F32R, kind="ExternalInput").ap()
    bf1_d = nc.dram_tensor("bf1", [DFF], F32, kind="ExternalInput").ap()
    wf2_d = nc.dram_tensor("wf2", [DFF, D], F32R, kind="ExternalInput").ap()
    bf2_d = nc.dram_tensor("bf2", [D], F32R, kind="ExternalInput").ap()
    out_d = nc.dram_tensor("out", [n_elems, L, D], F32, kind="ExternalOutput").ap()

    with SplitDrainTC(nc) as tc:
        _emit(nc, tc, n_elems, x_d, nidx_d, mask_d, btab_d, qT_d, w6_d,
              brows_d, bkrow_d, wf1_d, bf1_d, wf2_d, bf2_d, out_d)
    _split_excess_waits(nc, cap=1)
    return nc


def _layer_norm(nc, pool_small, xn, x_sb, eps_tile):
    """xn[:, lt, :] = (x - mean)/sqrt(var + LN_EPS). Tiles: [128, LT, 512]."""
    for lt in range(LT):
        stats = pool_small.tile([128, 6], F32, tag="ln_stats")
        nc.vector.bn_stats(stats[:], x_sb[:, lt, :])
        aggr = pool_small.tile([128, 2], F32, tag="ln_aggr")
        nc.vector.bn_aggr(aggr[:], stats[:])
        sv = pool_small.tile([128, 1], F32, tag="ln_sv")
        nc.scalar.activation(sv[:], aggr[:, 1:2], AF.Sqrt, bias=eps_tile[:])
        rstd = pool_small.tile([128, 1], F32, tag="ln_rstd")
        nc.vector.reciprocal(rstd[:], sv[:])
        nmr = pool_small.tile([128, 1], F32, tag="ln_nmr")
        nc.vector.tensor_scalar(out=nmr[:], in0=aggr[:, 0:1], scalar1=rstd[:],
                                scalar2=-1.0, op0=OP.mult, op1=OP.mult)
        nc.scalar.activation(xn[:, lt, :], x_sb[:, lt, :], AF.Identity,
                             bias=nmr[:], scale=rstd[:])


def _emit(nc, tc, n_elems, x_d, nidx_d, mask_d, btab_d, qT_d, w6_d, brows_d,
          bkrow_d, wf1_d, bf1_d, wf2_d, bf2_d, out_d):
    from contextlib import ExitStack

    top = ExitStack()
    with top:
        # ---- persistent constants ----
        const_pool = top.enter_context(tc.tile_pool(name="const", bufs=1))
        ident = const_pool.tile([128, 128], F32)
        from concourse.masks import make_identity
        make_identity(nc, ident[:])
        identr = const_pool.tile([128, 128], F32R)
        nc.vector.tensor_copy(identr[:], ident[:])
        ones_row = const_pool.tile([1, 128], F32R)
        nc.vector.memset(ones_row[:].bitcast(F32), 1.0)
        ones_col = const_pool.tile([128, 1], F32R)
        nc.vector.memset(ones_col[:].bitcast(F32), 1.0)
        eps_tile = const_pool.tile([128, 1], F32)
        nc.vector.memset(eps_tile[:], LN_EPS)
        bf2_sb = const_pool.tile([1, D], F32R)
        nc.scalar.dma_start(bf2_sb[:], bf2_d.rearrange("(o a) -> o a", o=1))
        bk_col = const_pool.tile([128, KD], F32)
        nc.scalar.dma_start(bk_col[:], bkrow_d.rearrange("(k p) -> p k", p=128))
        bf1_col = const_pool.tile([128, KF], F32)
        nc.scalar.dma_start(bf1_col[:], bf1_d.rearrange("(k p) -> p k", p=128))

        # ---- PSUM pools (shared across phases; 3+3+2 banks) ----
        ps_mm = top.enter_context(tc.tile_pool(name="ps_mm", bufs=3, space="PSUM"))
        ps_h = top.enter_context(tc.tile_pool(name="ps_h", bufs=3, space="PSUM"))
        ps_tr = top.enter_context(tc.tile_pool(name="ps_tr", bufs=2, space="PSUM"))

        small = top.enter_context(tc.tile_pool(name="small", bufs=2))

        # ---- DRAM scratch ----
        dram = top.enter_context(tc.tile_pool(name="dram", bufs=1, space="DRAM"))
        y2_dram = dram.tile([n_elems, L, D], F32)

        # ================= PHASE 1: static expansion =================
        with ExitStack() as p1:
            wpool = p1.enter_context(tc.tile_pool(name="w1", bufs=1))
            w6_sb = wpool.tile([128, 6, KD, D], F32R)
            nc.gpsimd.dma_start(w6_sb[:, 0, :, :],
                                w6_d[0].rearrange("(k p) n -> p k n", p=128))
            qT_sb = wpool.tile([128, KD, N], F32R)
            nc.gpsimd.dma_start(qT_sb[:], qT_d.rearrange("(k p) n -> p k n", p=128))
            for wi in range(1, 6):
                nc.gpsimd.dma_start(w6_sb[:, wi, :, :],
                                    w6_d[wi].rearrange("(k p) n -> p k n", p=128))

            a1 = p1.enter_context(tc.tile_pool(name="act1", bufs=1))
            a2 = p1.enter_context(tc.tile_pool(name="act1b", bufs=2))
            zdram_pool = p1.enter_context(
                tc.tile_pool(name="zdram", bufs=2, space="DRAM"))

            for b in range(n_elems):
                _phase1(nc, b, a1, a2, small, ps_mm, ps_h, ps_tr,
                        zdram_pool, x_d, nidx_d, mask_d, btab_d,
                        w6_sb, qT_sb, brows_d, bk_col, ident, identr,
                        ones_row, ones_col, eps_tile, y2_dram)

        # ================= PHASE 2: feed-forward =================
        with ExitStack() as p2:
            wpool2 = p2.enter_context(tc.tile_pool(name="w2", bufs=1))
            wf1_sb = wpool2.tile([128, KD, DFF], F32R)
            nc.gpsimd.dma_start(wf1_sb[:, :, :DFF // 2],
                                wf1_d[:, :DFF // 2].rearrange("(k p) n -> p k n", p=128))
            nc.gpsimd.dma_start(wf1_sb[:, :, DFF // 2:],
                                wf1_d[:, DFF // 2:].rearrange("(k p) n -> p k n", p=128))
            wf2_sb = wpool2.tile([128, KF, D], F32R)
            nc.gpsimd.dma_start(wf2_sb[:], wf2_d.rearrange("(k p) n -> p k n", p=128))

            f1 = p2.enter_context(tc.tile_pool(name="actf", bufs=2))

            for b in range(n_elems):
                _phase2(nc, b, f1, small, ps_mm, ps_h, wf1_sb,
                        wf2_sb, bf1_col, bf2_sb, ident, identr, ones_row,
                        eps_tile, y2_dram, out_d)


def _phase1(nc, b, a1, a2, small, ps_mm, ps_h, ps_tr, zdram_pool,
            x_d, nidx_d, mask_d, btab_d, w6_sb, qT_sb, brows_d, bk_col,
            ident, identr, ones_row, ones_col, eps_tile, y2_dram):
    W_K, W_A, W_GA, W_B, W_GB, W_S = range(6)

    # -- load x, idx, mask --
    x_sb = a2.tile([128, LT, D], F32, tag="x")
    nc.scalar.dma_start(x_sb[:], x_d[b].rearrange("(lt p) d -> p lt d", p=128))
    idx_sb = a2.tile([128, NMO], I32, tag="idx")
    nc.scalar.dma_start(idx_sb[:, 0:7],
                      nidx_d[b, 0:896].rearrange("(a p) -> p a", p=128))
    nc.scalar.dma_start(idx_sb[0:96, 7:8],
                      nidx_d[b, 896:992].rearrange("(a p) -> p a", p=96))
    maskf = a2.tile([128, NMO, L], F32, tag="maskf", bufs=1)
    for mo in range(NMO):
        m = NSZ[mo]
        mi32 = a2.tile([128, L], I32, tag="mi32", bufs=2)
        nc.scalar.dma_start(mi32[:m, :], mask_d[b, NOFF[mo]:NOFF[mo] + m, :])
        nc.vector.tensor_copy(maskf[:m, mo, :], mi32[:m, :])

    # -- LN1 --
    xn = a1.tile([128, LT, D], F32R, tag="xn")
    _layer_norm(nc, small, xn, x_sb, eps_tile)

    # -- x2T (transpose of xn): [128(d%128), KD, L] --
    x2T = a1.tile([128, KD, L], F32R, tag="x2T")
    for ko in range(KD):
        ps = ps_h.tile([128, L], F32, tag="h")
        for lt in range(LT):
            nc.tensor.transpose(_r(ps[:, lt * 128:(lt + 1) * 128]),
                                xn[:, lt, ko * 128:(ko + 1) * 128], identr[:])
        nc.scalar.copy(x2T[:, ko, :], ps[:])

    # -- x_keyT = (xn @ Wk')^T + bk' : [128, KD, L] --
    xkT = a1.tile([128, KD, L], F32R, tag="xkT")
    for ko in range(KD):
        ps = ps_h.tile([128, L], F32, tag="h")
        for ki in range(KD):
            nc.tensor.matmul(ps[:], _r(w6_sb[:, W_K, ki, ko * 128:(ko + 1) * 128]),
                             _r(x2T[:, ki, :]), start=(ki == 0), stop=(ki == KD - 1))
        nc.scalar.activation(xkT[:, ko, :], ps[:], AF.Identity,
                             bias=bk_col[:, ko:ko + 1])

    # -- zfull = q_tab_s @ x_keyT : [N, L], staged to DRAM for the row gather --
    zfull_dram = zdram_pool.tile([N, L], F32, tag="zfull")
    for mo in range(NMO):
        m = NSZ[mo]
        ps = ps_h.tile([128, L], F32, tag="h")
        for ki in range(KD):
            nc.tensor.matmul(ps[:m, :], _r(qT_sb[:, ki, NOFF[mo]:NOFF[mo] + m]),
                             _r(xkT[:, ki, :]), start=(ki == 0), stop=(ki == KD - 1))
        zst = a2.tile([128, L], F32, tag="zst", bufs=2)
        nc.scalar.copy(zst[:m, :], ps[:m, :])
        nc.sync.dma_start(zfull_dram[NOFF[mo]:NOFF[mo] + m, :], zst[:m, :])

    # -- gather z = zfull[idx]; bias_exp = b_tab[idx] --
    z_sb = a1.tile([128, NMO, L], F32, tag="z")
    bexp = a1.tile([128, NMO, D], F32, tag="bexp")
    for mo in range(NMO):
        m = NSZ[mo]
        nc.gpsimd.indirect_dma_start(
            out=z_sb[:m, mo, :], out_offset=None, in_=zfull_dram[:, :],
            in_offset=bass.IndirectOffsetOnAxis(ap=idx_sb[:m, mo:mo + 1], axis=0))
        nc.gpsimd.indirect_dma_start(
            out=bexp[:m, mo, :], out_offset=None, in_=btab_d[:, :],
            in_offset=bass.IndirectOffsetOnAxis(ap=idx_sb[:m, mo:mo + 1], axis=0))

    # -- az = relu(z)*m (+rowsums); bz' = min(z,0)*m = -relu(-z)*m --
    az = a1.tile([128, NMO, L], F32R, tag="az")
    bz = a1.tile([128, NMO, L], F32R, tag="bz")
    sum_a = small.tile([128, NMO], F32, tag="sum_a")
    sum_b = small.tile([128, NMO], F32, tag="sum_b")
    nc.vector.memset(sum_a[:], 1.0)
    nc.vector.memset(sum_b[:], 1.0)
    for mo in range(NMO):
        m = NSZ[mo]
        nc.vector.scalar_tensor_tensor(
            out=az[:m, mo, :], in0=z_sb[:m, mo, :], scalar=0.0,
            in1=maskf[:m, mo, :], op0=OP.max, op1=OP.mult,
            accum_out=sum_a[:m, mo:mo + 1])
        nc.vector.scalar_tensor_tensor(
            out=bz[:m, mo, :], in0=z_sb[:m, mo, :], scalar=0.0,
            in1=maskf[:m, mo, :], op0=OP.min, op1=OP.mult,
            accum_out=sum_b[:m, mo:mo + 1])
    rfw_a = small.tile([128, NMO], F32, tag="rfw_a")
    rfw_b = small.tile([128, NMO], F32, tag="rfw_b")
    tmp_a = small.tile([128, NMO], F32, tag="tmp_a")
    tmp_b = small.tile([128, NMO], F32, tag="tmp_b")
    nc.vector.tensor_scalar_add(tmp_a[:], sum_a[:], EPS)
    nc.vector.reciprocal(rfw_a[:], tmp_a[:])
    nc.vector.tensor_scalar_add(tmp_b[:], sum_b[:], -EPS)
    nc.vector.reciprocal(rfw_b[:], tmp_b[:])

    # -- a_emb / b_emb / sel (natural layout [128(l), LT, 512]) --
    def load_brow(bi):
        br = a2.tile([1, D], F32R, tag="brow", bufs=4, name=f"brow_{b}_{bi}")
        nc.scalar.dma_start(br[:], brows_d[bi].rearrange("(o a) -> o a", o=1))
        return br

    def gated_emb(wi, wgi, bi, bgi, tag):
        emb = a1.tile([128, LT, D], F32R, tag=tag)
        br_g = load_brow(bgi)
        br_a = load_brow(bi)
        for lt in range(LT):
            ps_g = ps_mm.tile([128, D], F32, tag="mm")
            for ki in range(KD):
                nc.tensor.matmul(ps_g[:], _r(x2T[:, ki, lt * 128:(lt + 1) * 128]),
                                 _r(w6_sb[:, wgi, ki, :]), start=(ki == 0),
                                 stop=False)
            nc.tensor.matmul(ps_g[:], _r(ones_row[:]), _r(br_g[:]),
                             start=False, stop=True)
            sig = a2.tile([128, D], F32, tag="sig", bufs=1)
            nc.scalar.activation(sig[:], ps_g[:], AF.Sigmoid)
            ps_a = ps_mm.tile([128, D], F32, tag="mm")
            for ki in range(KD):
                nc.tensor.matmul(ps_a[:], _r(x2T[:, ki, lt * 128:(lt + 1) * 128]),
                                 _r(w6_sb[:, wi, ki, :]), start=(ki == 0),
                                 stop=False)
            nc.tensor.matmul(ps_a[:], _r(ones_row[:]), _r(br_a[:]),
                             start=False, stop=True)
            nc.vector.tensor_tensor(out=emb[:, lt, :], in0=ps_a[:], in1=sig[:],
                                    op=OP.mult)
        return emb

    a_emb = gated_emb(W_A, W_GA, 1, 2, "a_emb")
    b_emb = gated_emb(W_B, W_GB, 3, 4, "b_emb")

    sel = a1.tile([128, LT, D], F32, tag="sel")
    br_s = load_brow(5)
    for lt in range(LT):
        ps_s = ps_mm.tile([128, D], F32, tag="mm")
        for ki in range(KD):
            nc.tensor.matmul(ps_s[:], _r(x2T[:, ki, lt * 128:(lt + 1) * 128]),
                             _r(w6_sb[:, W_S, ki, :]), start=(ki == 0), stop=False)
        nc.tensor.matmul(ps_s[:], _r(ones_row[:]), _r(br_s[:]),
                         start=False, stop=True)
        nc.scalar.activation(sel[:, lt, :], ps_s[:], AF.Sigmoid)

    # -- per side: fw weights (scaled+transposed), fw matmul, bw matmul --
    out_ab = []
    for side in range(2):
        zz = az if side == 0 else bz
        rfw = rfw_a if side == 0 else rfw_b
        emb = a_emb if side == 0 else b_emb

        fwT = a1.tile([128, LT, N], F32R, tag="fwT")
        for g in range(2):
            width = 512 if g == 0 else 480
            ps_lt = [ps_tr.tile([128, 512], F32, tag="tr", name=f"tr_{side}_{g}_{ltx}")
                     for ltx in range(LT)]
            for mi in range(4):
                mo = g * 4 + mi
                m = NSZ[mo]
                zs = a2.tile([128, L], F32R, tag="zs", bufs=3)
                nc.vector.tensor_scalar(out=zs[:m, :], in0=zz[:m, mo, :],
                                        scalar1=rfw[:m, mo:mo + 1],
                                        scalar2=None, op0=OP.mult)
                for lt in range(LT):
                    nc.tensor.transpose(_r(ps_lt[lt][:, mi * 128:mi * 128 + m]),
                                        zs[:m, lt * 128:(lt + 1) * 128],
                                        identr[:m, :m])
            for lt in range(LT):
                nc.scalar.copy(fwT[:, lt, g * 512:g * 512 + width],
                               ps_lt[lt][:, :width])

        # bw denominator: den[l] = sum_n zz[n, l]  (ones-col matmul -> [1, L]
        # row, then two tiny transposes -> per-partition [128, LT])
        drow = ps_h.tile([1, L], F32, tag="h")
        for mo in range(NMO):
            m = NSZ[mo]
            nc.tensor.matmul(drow[:1, :], _r(ones_col[:m, :]), _r(zz[:m, mo, :]),
                             start=(mo == 0), stop=(mo == NMO - 1))
        drow_sb = small.tile([1, L], F32, tag="drow", bufs=1)
        nc.scalar.copy(drow_sb[:], drow[:1, :])
        den_ps = ps_h.tile([128, LT], F32, tag="h")
        for lt in range(LT):
            nc.tensor.transpose(den_ps[:, lt:lt + 1],
                                drow_sb[:1, lt * 128:(lt + 1) * 128],
                                ident[:1, :1])
        rbw = small.tile([128, LT], F32, tag="rbw")
        tmp2 = small.tile([128, LT], F32, tag="tmp2")
        nc.vector.tensor_scalar_add(tmp2[:], den_ps[:],
                                    EPS if side == 0 else -EPS)
        nc.vector.reciprocal(rbw[:], tmp2[:])

        # fw matmul: cfw[n, d] = sum_l fwT[l, n]^T emb[l, d] + bexp
        cfw = a1.tile([128, NMO, D], F32R, tag="cfw")
        for mo in range(NMO):
            m = NSZ[mo]
            ps = ps_mm.tile([128, D], F32, tag="mm")
            for lt in range(LT):
                nc.tensor.matmul(ps[:m, :], _r(fwT[:, lt, NOFF[mo]:NOFF[mo] + m]),
                                 _r(emb[:, lt, :]), start=(lt == 0),
                                 stop=(lt == LT - 1))
            nc.vector.tensor_tensor(out=cfw[:m, mo, :], in0=ps[:m, :],
                                    in1=bexp[:m, mo, :], op=OP.add)

        # bw matmul: out[l, d] = rbw[l] * sum_n zz[n, l] cfw[n, d]
        out_raw = a1.tile([128, LT, D], F32, tag=f"out_{side}")
        for lt in range(LT):
            ps = ps_mm.tile([128, D], F32, tag="mm")
            for mo in range(NMO):
                m = NSZ[mo]
                nc.tensor.matmul(ps[:], _r(zz[:m, mo, lt * 128:(lt + 1) * 128]),
                                 _r(cfw[:m, mo, :]), start=(mo == 0),
                                 stop=(mo == NMO - 1))
            nc.scalar.activation(out_raw[:, lt, :], ps[:], AF.Copy,
                                 scale=rbw[:, lt:lt + 1])
        out_ab.append(out_raw)

    # -- combine: y2 = x + out_b + sel * (out_a - out_b) --
    out_a, out_b = out_ab
    y2 = a1.tile([128, LT, D], F32, tag="y2")
    for lt in range(LT):
        dt_ = a2.tile([128, D], F32, tag="cmb", bufs=2)
        nc.vector.tensor_tensor(out=dt_[:], in0=out_a[:, lt, :],
                                in1=out_b[:, lt, :], op=OP.subtract)
        mt = a2.tile([128, D], F32, tag="cmb", bufs=2)
        nc.vector.tensor_tensor(out=mt[:], in0=dt_[:], in1=sel[:, lt, :],
                                op=OP.mult)
        tt = a2.tile([128, D], F32, tag="cmb", bufs=2)
        nc.vector.tensor_tensor(out=tt[:], in0=x_sb[:, lt, :],
                                in1=out_b[:, lt, :], op=OP.add)
        nc.vector.tensor_tensor(out=y2[:, lt, :], in0=tt[:], in1=mt[:], op=OP.add)
    nc.sync.dma_start(y2_dram[b].rearrange("(lt p) d -> p lt d", p=128), y2[:])


def _phase2(nc, b, f1, small, ps_mm, ps_h, wf1_sb, wf2_sb,
            bf1_col, bf2_sb, ident, identr, ones_row, eps_tile, y2_dram, out_d):
    y2 = f1.tile([128, LT, D], F32, tag="y2p2")
    nc.scalar.dma_start(y2[:], y2_dram[b].rearrange("(lt p) d -> p lt d", p=128))
    x3 = f1.tile([128, LT, D], F32R, tag="x3")
    _layer_norm(nc, small, x3, y2, eps_tile)
    x3T = f1.tile([128, KD, L], F32R, tag="x3T")
    for ko in range(KD):
        ps = ps_h.tile([128, L], F32, tag="h")
        for lt in range(LT):
            nc.tensor.transpose(_r(ps[:, lt * 128:(lt + 1) * 128]),
                                x3[:, lt, ko * 128:(ko + 1) * 128], identr[:])
        nc.scalar.copy(x3T[:, ko, :], ps[:])

    hT = f1.tile([128, KF, L], F32R, tag="hT")
    for mo in range(KF):
        ps = ps_h.tile([128, L], F32, tag="h")
        for ki in range(KD):
            nc.tensor.matmul(ps[:], _r(wf1_sb[:, ki, mo * 128:(mo + 1) * 128]),
                             _r(x3T[:, ki, :]), start=(ki == 0), stop=(ki == KD - 1))
        nc.scalar.activation(hT[:, mo, :], ps[:], AF.Relu,
                             bias=bf1_col[:, mo:mo + 1])

    out_sb = f1.tile([128, LT, D], F32, tag="out_sb")
    for lt in range(LT):
        ps = ps_mm.tile([128, D], F32, tag="mm")
        for mo in range(KF):
            nc.tensor.matmul(ps[:], _r(hT[:, mo, lt * 128:(lt + 1) * 128]),
                             _r(wf2_sb[:, mo, :]), start=(mo == 0), stop=False)
        nc.tensor.matmul(ps[:], _r(ones_row[:]), _r(bf2_sb[:]),
                         start=False, stop=True)
        nc.vector.tensor_tensor(out=out_sb[:, lt, :], in0=ps[:],
                                in1=y2[:, lt, :], op=OP.add)
    nc.sync.dma_start(out_d[b].rearrange("(lt p) d -> p lt d", p=128), out_sb[:])


# ---------------------------------------------------------------------------
# host-side weight preprocessing + SPMD launch
# ---------------------------------------------------------------------------

def _prep_host(inputs):
    f = lambda k: np.ascontiguousarray(np.asarray(inputs[k], dtype=np.float32))
    g1, b1 = f("ln1_g"), f("ln1_b")
    g2, b2 = f("ln2_g"), f("ln2_b")
    Wk, bk = f("Wk"), f("bk")
    Wa, ba = f("Wa"), f("ba")
    Wa1, ba1 = f("Wa1"), f("ba1")
    Wb, bb = f("Wb"), f("bb")
    Wb1, bb1 = f("Wb1"), f("bb1")
    Ws, bsel = f("Ws"), f("bsel")
    Wf1, bf1 = f("Wf1"), f("bf1")
    Wf2, bf2 = f("Wf2"), f("bf2")
    q_tab, b_tab = f("q_tab"), f("b_tab")

    Waa1 = Wa @ Wa1
    Wbb1 = Wb @ Wb1
    w6 = np.stack([
        g1[:, None] * Wk,
        g1[:, None] * Wa,
        g1[:, None] * Waa1,
        g1[:, None] * Wb,
        g1[:, None] * Wbb1,
        g1[:, None] * Ws,
    ]).astype(np.float32)
    brows = np.stack([
        b1 @ Wk + bk,
        b1 @ Wa + ba,
        b1 @ Waa1 + ba @ Wa1 + ba1,
        b1 @ Wb + bb,
        b1 @ Wbb1 + bb @ Wb1 + bb1,
        b1 @ Ws + bsel,
    ]).astype(np.float32)
    qT = np.ascontiguousarray((q_tab / np.sqrt(np.float32(D))).T)
    wf1 = np.ascontiguousarray(g2[:, None] * Wf1)
    bf1p = (b2 @ Wf1 + bf1).astype(np.float32)
    return dict(b_tab=b_tab, qT=qT, w6=np.ascontiguousarray(w6),
                brows=np.ascontiguousarray(brows),
                bkrow=np.ascontiguousarray(brows[0]), wf1=wf1, bf1=bf1p,
                wf2=Wf2, bf2=bf2)


_NC_CACHE = {}


def _get_program(n_elems=BPC):
    if n_elems not in _NC_CACHE:
        _NC_CACHE[n_elems] = build_program(n_elems)
    return _NC_CACHE[n_elems]


def make_in_maps(inputs):
    x = np.ascontiguousarray(np.asarray(inputs["x"], dtype=np.float32))
    nidx = np.ascontiguousarray(np.asarray(inputs["n_indexes"]).astype(np.int32))
    mask = np.ascontiguousarray(np.asarray(inputs["mask"]).astype(np.int32))
    shared = _prep_host(inputs)
    in_maps = []
    for c in range(NCORES):
        sl = slice(c * BPC, (c + 1) * BPC)
        in_maps.append({
            "x": np.ascontiguousarray(x[sl]),
            "nidx": np.ascontiguousarray(nidx[sl]),
            "mask": np.ascontiguousarray(mask[sl]),
            **shared,
        })
    return in_maps


def kernel(**inputs):
    from concourse.bass_utils import run_bass_kernel_spmd

    nc = _get_program(BPC)
    in_maps = make_in_maps(inputs)
    res = run_bass_kernel_spmd(nc, in_maps, core_ids=list(range(NCORES)))
    out = np.concatenate([res.results[c]["out"] for c in range(NCORES)], axis=0)
    return out.astype(np.float32)


# revision 21
# speedup vs baseline: 1.0184x; 1.0184x over previous
"""Trainium2 Bass kernel for nn_EncoderLayer (dense transformer encoder layer
with static-expansion attention-like block + FF), data-parallel over 8 cores.

Contract: kernel(**inputs) takes FULL unsharded inputs (as in setup_inputs()),
returns the FULL (64, 256, 512) float32 output.
"""

import sys

for _p in ("/opt/trn_rl_repo",):
    if _p not in sys.path:
        sys.path.insert(0, _p)

import numpy as np

import concourse.bass as bass
import concourse.mybir as mybir
import concourse.tile as tile
from concourse.vector_clock import ScopedClock

F32 = mybir.dt.float32
F32R = mybir.dt.float32r
I32 = mybir.dt.int32
AX = mybir.AxisListType
OP = mybir.AluOpType
AF = mybir.ActivationFunctionType

D = 512          # d_model
DFF = 2048       # d_ff
N = 992          # n experts
L = 256          # enc len
BS = 64
NCORES = 8
BPC = BS // NCORES  # batch elements per core
EPS = 1e-9
LN_EPS = 1e-5

KD = D // 128     # 4 k-chunks over d_model
LT = L // 128     # 2 l-chunks
NMO = 8           # n-chunks over N (7x128 + 96)
NSZ = [128] * 7 + [96]
NOFF = [128 * i for i in range(8)]
KF = DFF // 128   # 16 chunks over d_ff


class SplitDrainTC(tile.TileContext):
    """TileContext whose exit drain splits semaphore waits across nop
    instructions (this walrus build rejects >2 sync waits on one Drain)."""

    def _drain_and_barrier(self, tick_clock, wait_clock):
        nc = self.nc
        probe = nc.sync.nop(nofuse=True)
        wait_clock.add_sem_waits(probe.ins, ScopedClock({None: tick_clock.global_clock}))
        si = probe.ins.sync_info
        waits = list(si.on_wait) if si and si.on_wait else []
        if len(waits) > 1:
            si.on_wait = waits[:1]
            sems_by_name = {h.name: h for h in self.sems.allocated().values()}
            for w in waits[1:]:
                n2 = nc.sync.nop(nofuse=True)
                n2.wait_op(sems_by_name[w.ant_name], w.wait_value, "sem-ge")
        nc.sync.drain()
        nc.all_engine_barrier()
        popped = nc._tile_sem_poison_stack.pop()
        assert popped is self._sem_poison
        nc.clear_and_free_semaphores(list(self.sems.allocated().values()))
        nc.all_engine_barrier()


def _r(ap):
    return ap.bitcast(F32R)


def _split_excess_waits(nc, cap=2):
    """This walrus build rejects instructions carrying more than ~2 sync
    waits. Hoist excess waits onto same-engine nop instructions inserted
    immediately before the offending instruction (engine program order is
    bb order, so the nop's waits complete first)."""
    import bass_rust
    for f in nc.m.functions:
        for bb in f.blocks:
            over = [inst for inst in bb.instructions
                    if inst.sync_info and inst.sync_info.on_wait
                    and len(inst.sync_info.on_wait) > cap]
            if not over:
                continue
            carriers = {}
            for inst in over:
                waits = list(inst.sync_info.on_wait)
                inst.sync_info.on_wait = waits[:cap]
                rest = waits[cap:]
                lst = []
                for i in range(0, len(rest), cap):
                    nop = nc.engines[inst.engine].nop(nofuse=True)
                    cur = nc.cur_bb.bb
                    assert cur.instructions[-1] is nop.ins
                    cur.instructions.pop()
                    nop.ins.sync_info = bass_rust.SyncInfo(
                        on_wait=rest[i:i + cap], on_update=[])
                    lst.append(nop.ins)
                carriers[inst.name] = lst
            out = []
            for inst in bb.instructions:
                out.extend(carriers.get(inst.name, ()))
                out.append(inst)
            bb.instructions[:] = out


def build_program(n_elems=BPC):
    """Single-core SPMD program; see kernel() for the per-core input map."""
    nc = bass.Bass("TRN2", target_bir_lowering=False, debug=False)

    x_d = nc.dram_tensor("x", [n_elems, L, D], F32, kind="ExternalInput").ap()
    nidx_d = nc.dram_tensor("nidx", [n_elems, N], I32, kind="ExternalInput").ap()
    mask_d = nc.dram_tensor("mask", [n_elems, N, L], I32, kind="ExternalInput").ap()
    btab_d = nc.dram_tensor("b_tab", [N, D], F32, kind="ExternalInput").ap()
    qT_d = nc.dram_tensor("qT", [D, N], F32R, kind="ExternalInput").ap()
    w6_d = nc.dram_tensor("w6", [6, D, D], F32R, kind="ExternalInput").ap()
    brows_d = nc.dram_tensor("brows", [6, D], F32R, kind="ExternalInput").ap()
    bkrow_d = nc.dram_tensor("bkrow", [D], F32, kind="ExternalInput").ap()
    wf1_d = nc.dram_tensor("wf1", [D, DFF], F32R, kind="ExternalInput").ap()
    bf1_d = nc.dram_tensor("bf1", [DFF], F32, kind="ExternalInput").ap()
    wf2_d = nc.dram_tensor("wf2", [DFF, D], F32R, kind="ExternalInput").ap()
    bf2_d = nc.dram_tensor("bf2", [D], F32R, kind="ExternalInput").ap()
    out_d = nc.dram_tensor("out", [n_elems, L, D], F32, kind="ExternalOutput").ap()

    with SplitDrainTC(nc) as tc:
        _emit(nc, tc, n_elems, x_d, nidx_d, mask_d, btab_d, qT_d, w6_d,
              brows_d, bkrow_d, wf1_d, bf1_d, wf2_d, bf2_d, out_d)
    _split_excess_waits(nc, cap=1)
    return nc


def _layer_norm(nc, pool_small, xn, x_sb, eps_tile):
    """xn[:, lt, :] = (x - mean)/sqrt(var + LN_EPS). Tiles: [128, LT, 512]."""
    for lt in range(LT):
        stats = pool_small.tile([128, 6], F32, tag="ln_stats")
        nc.vector.bn_stats(stats[:], x_sb[:, lt, :])
        aggr = pool_small.tile([128, 2], F32, tag="ln_aggr")
        nc.vector.bn_aggr(aggr[:], stats[:])
        sv = pool_small.tile([128, 1], F32, tag="ln_sv")
        nc.scalar.activation(sv[:], aggr[:, 1:2], AF.Sqrt, bias=eps_tile[:])
        rstd = pool_small.tile([128, 1], F32, tag="ln_rstd")
        nc.vector.reciprocal(rstd[:], sv[:])
        nmr = pool_small.tile([128, 1], F32, tag="ln_nmr")
        nc.vector.tensor_scalar(out=nmr[:], in0=aggr[:, 0:1], scalar1=rstd[:],
                                scalar2=-1.0, op0=OP.mult, op1=OP.mult)
        nc.scalar.activation(xn[:, lt, :], x_sb[:, lt, :], AF.Identity,
                             bias=nmr[:], scale=rstd[:])


def _emit(nc, tc, n_elems, x_d, nidx_d, mask_d, btab_d, qT_d, w6_d, brows_d,
          bkrow_d, wf1_d, bf1_d, wf2_d, bf2_d, out_d):
    from contextlib import ExitStack

    top = ExitStack()
    with top:
        # ---- persistent constants ----
        const_pool = top.enter_context(tc.tile_pool(name="const", bufs=1))
        ident = const_pool.tile([128, 128], F32)
        from concourse.masks import make_identity
        make_identity(nc, ident[:])
        identr = const_pool.tile([128, 128], F32R)
        nc.vector.tensor_copy(identr[:], ident[:])
        ones_row = const_pool.tile([1, 128], F32R)
        nc.vector.memset(ones_row[:].bitcast(F32), 1.0)
        ones_col = const_pool.tile([128, 1], F32R)
        nc.vector.memset(ones_col[:].bitcast(F32), 1.0)
        eps_tile = const_pool.tile([128, 1], F32)
        nc.vector.memset(eps_tile[:], LN_EPS)
        bf2_sb = const_pool.tile([1, D], F32R)
        nc.sync.dma_start(bf2_sb[:], bf2_d.rearrange("(o a) -> o a", o=1))
        bk_col = const_pool.tile([128, KD], F32)
        nc.sync.dma_start(bk_col[:], bkrow_d.rearrange("(k p) -> p k", p=128))
        bf1_col = const_pool.tile([128, KF], F32)
        nc.sync.dma_start(bf1_col[:], bf1_d.rearrange("(k p) -> p k", p=128))

        # ---- PSUM pools (shared across phases; 3+3+2 banks) ----
        ps_mm = top.enter_context(tc.tile_pool(name="ps_mm", bufs=3, space="PSUM"))
        ps_h = top.enter_context(tc.tile_pool(name="ps_h", bufs=3, space="PSUM"))
        ps_tr = top.enter_context(tc.tile_pool(name="ps_tr", bufs=2, space="PSUM"))

        small = top.enter_context(tc.tile_pool(name="small", bufs=2))

        # ---- DRAM scratch ----
        dram = top.enter_context(tc.tile_pool(name="dram", bufs=1, space="DRAM"))
        y2_dram = dram.tile([n_elems, L, D], F32)

        # ================= PHASE 1: static expansion =================
        with ExitStack() as p1:
            wpool = p1.enter_context(tc.tile_pool(name="w1", bufs=1))
            w6_sb = wpool.tile([128, 6, KD, D], F32R)
            nc.gpsimd.dma_start(w6_sb[:, 0, :, :],
                                w6_d[0].rearrange("(k p) n -> p k n", p=128))
            qT_sb = wpool.tile([128, KD, N], F32R)
            nc.gpsimd.dma_start(qT_sb[:], qT_d.rearrange("(k p) n -> p k n", p=128))
            for wi in range(1, 6):
                nc.gpsimd.dma_start(w6_sb[:, wi, :, :],
                                    w6_d[wi].rearrange("(k p) n -> p k n", p=128))

            a1 = p1.enter_context(tc.tile_pool(name="act1", bufs=1))
            a2 = p1.enter_context(tc.tile_pool(name="act1b", bufs=2))
            zdram_pool = p1.enter_context(
                tc.tile_pool(name="zdram", bufs=2, space="DRAM"))

            for b in range(n_elems):
                _phase1(nc, b, a1, a2, small, ps_mm, ps_h, ps_tr,
                        zdram_pool, x_d, nidx_d, mask_d, btab_d,
                        w6_sb, qT_sb, brows_d, bk_col, ident, identr,
                        ones_row, ones_col, eps_tile, y2_dram)

        # ================= PHASE 2: feed-forward =================
        with ExitStack() as p2:
            wpool2 = p2.enter_context(tc.tile_pool(name="w2", bufs=1))
            wf1_sb = wpool2.tile([128, KD, DFF], F32R)
            nc.gpsimd.dma_start(wf1_sb[:, :, :DFF // 2],
                                wf1_d[:, :DFF // 2].rearrange("(k p) n -> p k n", p=128))
            nc.gpsimd.dma_start(wf1_sb[:, :, DFF // 2:],
                                wf1_d[:, DFF // 2:].rearrange("(k p) n -> p k n", p=128))
            wf2_sb = wpool2.tile([128, KF, D], F32R)
            nc.gpsimd.dma_start(wf2_sb[:], wf2_d.rearrange("(k p) n -> p k n", p=128))

            f1 = p2.enter_context(tc.tile_pool(name="actf", bufs=2))

            for b in range(n_elems):
                _phase2(nc, b, f1, small, ps_mm, ps_h, wf1_sb,
                        wf2_sb, bf1_col, bf2_sb, ident, identr, ones_row,
                        eps_tile, y2_dram, out_d)


def _phase1(nc, b, a1, a2, small, ps_mm, ps_h, ps_tr, zdram_pool,
            x_d, nidx_d, mask_d, btab_d, w6_sb, qT_sb, brows_d, bk_col,
            ident, identr, ones_row, ones_col, eps_tile, y2_dram):
    W_K, W_A, W_GA, W_B, W_GB, W_S = range(6)

    # -- load x, idx, mask --
    x_sb = a2.tile([128, LT, D], F32, tag="x")
    nc.sync.dma_start(x_sb[:], x_d[b].rearrange("(lt p) d -> p lt d", p=128))
    idx_sb = a2.tile([128, NMO], I32, tag="idx")
    nc.sync.dma_start(idx_sb[:, 0:7],
                      nidx_d[b, 0:896].rearrange("(a p) -> p a", p=128))
    nc.sync.dma_start(idx_sb[0:96, 7:8],
                      nidx_d[b, 896:992].rearrange("(a p) -> p a", p=96))
    maskf = a2.tile([128, NMO, L], F32, tag="maskf", bufs=1)
    for mo in range(NMO):
        m = NSZ[mo]
        mi32 = a2.tile([128, L], I32, tag="mi32", bufs=2)
        nc.sync.dma_start(mi32[:m, :], mask_d[b, NOFF[mo]:NOFF[mo] + m, :])
        nc.gpsimd.tensor_copy(maskf[:m, mo, :], mi32[:m, :])

    # -- LN1 --
    xn = a1.tile([128, LT, D], F32R, tag="xn")
    _layer_norm(nc, small, xn, x_sb, eps_tile)

    # -- x2T (transpose of xn): [128(d%128), KD, L] --
    x2T = a1.tile([128, KD, L], F32R, tag="x2T")
    for ko in range(KD):
        ps = ps_h.tile([128, L], F32, tag="h")
        for lt in range(LT):
            nc.tensor.transpose(_r(ps[:, lt * 128:(lt + 1) * 128]),
                                xn[:, lt, ko * 128:(ko + 1) * 128], identr[:])
        nc.scalar.copy(x2T[:, ko, :], ps[:])

    # -- x_keyT = (xn @ Wk')^T + bk' : [128, KD, L] --
    xkT = a1.tile([128, KD, L], F32R, tag="xkT")
    for ko in range(KD):
        ps = ps_h.tile([128, L], F32, tag="h")
        for ki in range(KD):
            nc.tensor.matmul(ps[:], _r(w6_sb[:, W_K, ki, ko * 128:(ko + 1) * 128]),
                             _r(x2T[:, ki, :]), start=(ki == 0), stop=(ki == KD - 1))
        nc.scalar.activation(xkT[:, ko, :], ps[:], AF.Identity,
                             bias=bk_col[:, ko:ko + 1])

    # -- zfull = q_tab_s @ x_keyT : [N, L], staged to DRAM for the row gather --
    zfull_dram = zdram_pool.tile([N, L], F32, tag="zfull")
    for mo in range(NMO):
        m = NSZ[mo]
        ps = ps_h.tile([128, L], F32, tag="h")
        for ki in range(KD):
            nc.tensor.matmul(ps[:m, :], _r(qT_sb[:, ki, NOFF[mo]:NOFF[mo] + m]),
                             _r(xkT[:, ki, :]), start=(ki == 0), stop=(ki == KD - 1))
        zst = a2.tile([128, L], F32, tag="zst", bufs=2)
        nc.scalar.copy(zst[:m, :], ps[:m, :])
        nc.sync.dma_start(zfull_dram[NOFF[mo]:NOFF[mo] + m, :], zst[:m, :])

    # -- gather z = zfull[idx]; bias_exp = b_tab[idx] --
    z_sb = a1.tile([128, NMO, L], F32, tag="z")
    bexp = a1.tile([128, NMO, D], F32, tag="bexp")
    for mo in range(NMO):
        m = NSZ[mo]
        nc.gpsimd.indirect_dma_start(
            out=z_sb[:m, mo, :], out_offset=None, in_=zfull_dram[:, :],
            in_offset=bass.IndirectOffsetOnAxis(ap=idx_sb[:m, mo:mo + 1], axis=0))
        nc.gpsimd.indirect_dma_start(
            out=bexp[:m, mo, :], out_offset=None, in_=btab_d[:, :],
            in_offset=bass.IndirectOffsetOnAxis(ap=idx_sb[:m, mo:mo + 1], axis=0))

    # -- az = relu(z)*m (+rowsums); bz' = min(z,0)*m = -relu(-z)*m --
    az = a1.tile([128, NMO, L], F32R, tag="az")
    bz = a1.tile([128, NMO, L], F32R, tag="bz")
    sum_a = small.tile([128, NMO], F32, tag="sum_a")
    sum_b = small.tile([128, NMO], F32, tag="sum_b")
    nc.vector.memset(sum_a[:], 1.0)
    nc.vector.memset(sum_b[:], 1.0)
    for mo in range(NMO):
        m = NSZ[mo]
        nc.vector.scalar_tensor_tensor(
            out=az[:m, mo, :], in0=z_sb[:m, mo, :], scalar=0.0,
            in1=maskf[:m, mo, :], op0=OP.max, op1=OP.mult,
            accum_out=sum_a[:m, mo:mo + 1])
        nc.vector.scalar_tensor_tensor(
            out=bz[:m, mo, :], in0=z_sb[:m, mo, :], scalar=0.0,
            in1=maskf[:m, mo, :], op0=OP.min, op1=OP.mult,
            accum_out=sum_b[:m, mo:mo + 1])
    rfw_a = small.tile([128, NMO], F32, tag="rfw_a")
    rfw_b = small.tile([128, NMO], F32, tag="rfw_b")
    tmp_a = small.tile([128, NMO], F32, tag="tmp_a")
    tmp_b = small.tile([128, NMO], F32, tag="tmp_b")
    nc.vector.tensor_scalar_add(tmp_a[:], sum_a[:], EPS)
    nc.vector.reciprocal(rfw_a[:], tmp_a[:])
    nc.vector.tensor_scalar_add(tmp_b[:], sum_b[:], -EPS)
    nc.vector.reciprocal(rfw_b[:], tmp_b[:])

    # -- a_emb / b_emb / sel (natural layout [128(l), LT, 512]) --
    def load_brow(bi):
        br = a2.tile([1, D], F32R, tag="brow", bufs=4, name=f"brow_{b}_{bi}")
        nc.sync.dma_start(br[:], brows_d[bi].rearrange("(o a) -> o a", o=1))
        return br

    def gated_emb(wi, wgi, bi, bgi, tag):
        emb = a1.tile([128, LT, D], F32R, tag=tag)
        br_g = load_brow(bgi)
        br_a = load_brow(bi)
        for lt in range(LT):
            ps_g = ps_mm.tile([128, D], F32, tag="mm")
            for ki in range(KD):
                nc.tensor.matmul(ps_g[:], _r(x2T[:, ki, lt * 128:(lt + 1) * 128]),
                                 _r(w6_sb[:, wgi, ki, :]), start=(ki == 0),
                                 stop=False)
            nc.tensor.matmul(ps_g[:], _r(ones_row[:]), _r(br_g[:]),
                             start=False, stop=True)
            sig = a2.tile([128, D], F32, tag="sig", bufs=1)
            nc.scalar.activation(sig[:], ps_g[:], AF.Sigmoid)
            ps_a = ps_mm.tile([128, D], F32, tag="mm")
            for ki in range(KD):
                nc.tensor.matmul(ps_a[:], _r(x2T[:, ki, lt * 128:(lt + 1) * 128]),
                                 _r(w6_sb[:, wi, ki, :]), start=(ki == 0),
                                 stop=False)
            nc.tensor.matmul(ps_a[:], _r(ones_row[:]), _r(br_a[:]),
                             start=False, stop=True)
            nc.vector.tensor_tensor(out=emb[:, lt, :], in0=ps_a[:], in1=sig[:],
                                    op=OP.mult)
        return emb

    a_emb = gated_emb(W_A, W_GA, 1, 2, "a_emb")
    b_emb = gated_emb(W_B, W_GB, 3, 4, "b_emb")

    sel = a1.tile([128, LT, D], F32, tag="sel")
    br_s = load_brow(5)
    for lt in range(LT):
        ps_s = ps_mm.tile([128, D], F32, tag="mm")
        for ki in range(KD):
            nc.tensor.matmul(ps_s[:], _r(x2T[:, ki, lt * 128:(lt + 1) * 128]),
                             _r(w6_sb[:, W_S, ki, :]), start=(ki == 0), stop=False)
        nc.tensor.matmul(ps_s[:], _r(ones_row[:]), _r(br_s[:]),
                         start=False, stop=True)
        nc.scalar.activation(sel[:, lt, :], ps_s[:], AF.Sigmoid)

    # -- per side: fw weights (scaled+transposed), fw matmul, bw matmul --
    out_ab = []
    for side in range(2):
        zz = az if side == 0 else bz
        rfw = rfw_a if side == 0 else rfw_b
        emb = a_emb if side == 0 else b_emb

        fwT = a1.tile([128, LT, N], F32R, tag="fwT")
        for g in range(2):
            width = 512 if g == 0 else 480
            ps_lt = [ps_tr.tile([128, 512], F32, tag="tr", name=f"tr_{side}_{g}_{ltx}")
                     for ltx in range(LT)]
            for mi in range(4):
                mo = g * 4 + mi
                m = NSZ[mo]
                zs = a2.tile([128, L], F32R, tag="zs", bufs=3)
                nc.vector.tensor_scalar(out=zs[:m, :], in0=zz[:m, mo, :],
                                        scalar1=rfw[:m, mo:mo + 1],
                                        scalar2=None, op0=OP.mult)
                for lt in range(LT):
                    nc.tensor.transpose(_r(ps_lt[lt][:, mi * 128:mi * 128 + m]),
                                        zs[:m, lt * 128:(lt + 1) * 128],
                                        identr[:m, :m])
            for lt in range(LT):
                nc.scalar.copy(fwT[:, lt, g * 512:g * 512 + width],
                               ps_lt[lt][:, :width])

        # bw denominator: den[l] = sum_n zz[n, l]  (ones-col matmul -> [1, L]
        # row, then two tiny transposes -> per-partition [128, LT])
        drow = ps_h.tile([1, L], F32, tag="h")
        for mo in range(NMO):
            m = NSZ[mo]
            nc.tensor.matmul(drow[:1, :], _r(ones_col[:m, :]), _r(zz[:m, mo, :]),
                             start=(mo == 0), stop=(mo == NMO - 1))
        drow_sb = small.tile([1, L], F32, tag="drow", bufs=1)
        nc.scalar.copy(drow_sb[:], drow[:1, :])
        den_ps = ps_h.tile([128, LT], F32, tag="h")
        for lt in range(LT):
            nc.tensor.transpose(den_ps[:, lt:lt + 1],
                                drow_sb[:1, lt * 128:(lt + 1) * 128],
                                ident[:1, :1])
        rbw = small.tile([128, LT], F32, tag="rbw")
        tmp2 = small.tile([128, LT], F32, tag="tmp2")
        nc.vector.tensor_scalar_add(tmp2[:], den_ps[:],
                                    EPS if side == 0 else -EPS)
        nc.vector.reciprocal(rbw[:], tmp2[:])

        # fw matmul: cfw[n, d] = sum_l fwT[l, n]^T emb[l, d] + bexp
        cfw = a1.tile([128, NMO, D], F32R, tag="cfw")
        for mo in range(NMO):
            m = NSZ[mo]
            ps = ps_mm.tile([128, D], F32, tag="mm")
            for lt in range(LT):
                nc.tensor.matmul(ps[:m, :], _r(fwT[:, lt, NOFF[mo]:NOFF[mo] + m]),
                                 _r(emb[:, lt, :]), start=(lt == 0),
                                 stop=(lt == LT - 1))
            nc.vector.tensor_tensor(out=cfw[:m, mo, :], in0=ps[:m, :],
                                    in1=bexp[:m, mo, :], op=OP.add)

        # bw matmul: out[l, d] = rbw[l] * sum_n zz[n, l] cfw[n, d]
        out_raw = a1.tile([128, LT, D], F32, tag=f"out_{side}")
        for lt in range(LT):
            ps = ps_mm.tile([128, D], F32, tag="mm")
            for mo in range(NMO):
                m = NSZ[mo]
                nc.tensor.matmul(ps[:], _r(zz[:m, mo, lt * 128:(lt + 1) * 128]),
                                 _r(cfw[:m, mo, :]), start=(mo == 0),
                                 stop=(mo == NMO - 1))
            nc.scalar.activation(out_raw[:, lt, :], ps[:], AF.Copy,
                                 scale=rbw[:, lt:lt + 1])
        out_ab.append(out_raw)

    # -- combine: y2 = x + out_b + sel * (out_a - out_b) --
    out_a, out_b = out_ab
    y2 = a1.tile([128, LT, D], F32, tag="y2")
    for lt in range(LT):
        dt_ = a2.tile([128, D], F32, tag="cmb", bufs=2)
        nc.vector.tensor_tensor(out=dt_[:], in0=out_a[:, lt, :],
                                in1=out_b[:, lt, :], op=OP.subtract)
        mt = a2.tile([128, D], F32, tag="cmb", bufs=2)
        nc.vector.tensor_tensor(out=mt[:], in0=dt_[:], in1=sel[:, lt, :],
                                op=OP.mult)
        tt = a2.tile([128, D], F32, tag="cmb", bufs=2)
        nc.vector.tensor_tensor(out=tt[:], in0=x_sb[:, lt, :],
                                in1=out_b[:, lt, :], op=OP.add)
        nc.vector.tensor_tensor(out=y2[:, lt, :], in0=tt[:], in1=mt[:], op=OP.add)
    nc.sync.dma_start(y2_dram[b].rearrange("(lt p) d -> p lt d", p=128), y2[:])


def _phase2(nc, b, f1, small, ps_mm, ps_h, wf1_sb, wf2_sb,
            bf1_col, bf2_sb, ident, identr, ones_row, eps_tile, y2_dram, out_d):
    y2 = f1.tile([128, LT, D], F32, tag="y2p2")
    nc.sync.dma_start(y2[:], y2_dram[b].rearrange("(lt p) d -> p lt d", p=128))
    x3 = f1.tile([128, LT, D], F32R, tag="x3")
    _layer_norm(nc, small, x3, y2, eps_tile)
    x3T = f1.tile([128, KD, L], F32R, tag="x3T")
    for ko in range(KD):
        ps = ps_h.tile([128, L], F32, tag="h")
        for lt in range(LT):
            nc.tensor.transpose(_r(ps[:, lt * 128:(lt + 1) * 128]),
                                x3[:, lt, ko * 128:(ko + 1) * 128], identr[:])
        nc.scalar.copy(x3T[:, ko, :], ps[:])

    hT = f1.tile([128, KF, L], F32R, tag="hT")
    for mo in range(KF):
        ps = ps_h.tile([128, L], F32, tag="h")
        for ki in range(KD):
            nc.tensor.matmul(ps[:], _r(wf1_sb[:, ki, mo * 128:(mo + 1) * 128]),
                             _r(x3T[:, ki, :]), start=(ki == 0), stop=(ki == KD - 1))
        nc.scalar.activation(hT[:, mo, :], ps[:], AF.Relu,
                             bias=bf1_col[:, mo:mo + 1])

    out_sb = f1.tile([128, LT, D], F32, tag="out_sb")
    for lt in range(LT):
        ps = ps_mm.tile([128, D], F32, tag="mm")
        for mo in range(KF):
            nc.tensor.matmul(ps[:], _r(hT[:, mo, lt * 128:(lt + 1) * 128]),
                             _r(wf2_sb[:, mo, :]), start=(mo == 0), stop=False)
        nc.tensor.matmul(ps[:], _r(ones_row[:]), _r(bf2_sb[:]),
                         start=False, stop=True)
        nc.vector.tensor_tensor(out=out_sb[:, lt, :], in0=ps[:],
                                in1=y2[:, lt, :], op=OP.add)
    nc.sync.dma_start(out_d[b].rearrange("(lt p) d -> p lt d", p=128), out_sb[:])


# ---------------------------------------------------------------------------
# host-side weight preprocessing + SPMD launch
# ---------------------------------------------------------------------------

def _prep_host(inputs):
    f = lambda k: np.ascontiguousarray(np.asarray(inputs[k], dtype=np.float32))
    g1, b1 = f("ln1_g"), f("ln1_b")
    g2, b2 = f("ln2_g"), f("ln2_b")
    Wk, bk = f("Wk"), f("bk")
    Wa, ba = f("Wa"), f("ba")
    Wa1, ba1 = f("Wa1"), f("ba1")
    Wb, bb = f("Wb"), f("bb")
    Wb1, bb1 = f("Wb1"), f("bb1")
    Ws, bsel = f("Ws"), f("bsel")
    Wf1, bf1 = f("Wf1"), f("bf1")
    Wf2, bf2 = f("Wf2"), f("bf2")
    q_tab, b_tab = f("q_tab"), f("b_tab")

    Waa1 = Wa @ Wa1
    Wbb1 = Wb @ Wb1
    w6 = np.stack([
        g1[:, None] * Wk,
        g1[:, None] * Wa,
        g1[:, None] * Waa1,
        g1[:, None] * Wb,
        g1[:, None] * Wbb1,
        g1[:, None] * Ws,
    ]).astype(np.float32)
    brows = np.stack([
        b1 @ Wk + bk,
        b1 @ Wa + ba,
        b1 @ Waa1 + ba @ Wa1 + ba1,
        b1 @ Wb + bb,
        b1 @ Wbb1 + bb @ Wb1 + bb1,
        b1 @ Ws + bsel,
    ]).astype(np.float32)
    qT = np.ascontiguousarray((q_tab / np.sqrt(np.float32(D))).T)
    wf1 = np.ascontiguousarray(g2[:, None] * Wf1)
    bf1p = (b2 @ Wf1 + bf1).astype(np.float32)
    return dict(b_tab=b_tab, qT=qT, w6=np.ascontiguousarray(w6),
                brows=np.ascontiguousarray(brows),
                bkrow=np.ascontiguousarray(brows[0]), wf1=wf1, bf1=bf1p,
                wf2=Wf2, bf2=bf2)


_NC_CACHE = {}


def _get_program(n_elems=BPC):
    if n_elems not in _NC_CACHE:
        _NC_CACHE[n_elems] = build_program(n_elems)
    return _NC_CACHE[n_elems]


def make_in_maps(inputs):
    x = np.ascontiguousarray(np.asarray(inputs["x"], dtype=np.float32))
    nidx = np.ascontiguousarray(np.asarray(inputs["n_indexes"]).astype(np.int32))
    mask = np.ascontiguousarray(np.asarray(inputs["mask"]).astype(np.int32))
    shared = _prep_host(inputs)
    in_maps = []
    for c in range(NCORES):
        sl = slice(c * BPC, (c + 1) * BPC)
        in_maps.append({
            "x": np.ascontiguousarray(x[sl]),
            "nidx": np.ascontiguousarray(nidx[sl]),
            "mask": np.ascontiguousarray(mask[sl]),
            **shared,
        })
    return in_maps


def kernel(**inputs):
    from concourse.bass_utils import run_bass_kernel_spmd

    nc = _get_program(BPC)
    in_maps = make_in_maps(inputs)
    res = run_bass_kernel_spmd(nc, in_maps, core_ids=list(range(NCORES)))
    out = np.concatenate([res.results[c]["out"] for c in range(NCORES)], axis=0)
    return out.astype(np.float32)
